# revision 4
# baseline (speedup 1.0000x reference)
"""Trainium2 Bass kernel for the 2-layer GRU problem (nn_GRU_43568148251487).

Contract: kernel(**inputs) takes the FULL unsharded inputs (batch 64) and
returns the FULL output [64, 512, 64]. Internally: data-parallel over batch
across 8 NeuronCores (8 sequences per core), GRU weights replicated; one
SPMD Bass program, no collectives.

Per-core structure (T=512, H=512, G=3H=1536, B=8):
  phase A: gi0 = x @ (W_ih0 Wx).T + t @ (W_ih0 Wt).T + bias  (host-fused
           input projection), bulk matmuls -> DRAM.
  phase B: 544 "supersteps": layer0 runs steps 0..511, layer1 lags by 32.
           Per step and layer: gates = h @ W_hh.T as 12 accumulating f32r
           matmuls (lhsT = transposed hidden state [128,8] chunks, rhs =
           W_hh.T [128,512] chunks streaming at 1 cycle/row), elementwise
           GRU cell on [8,512] tiles split across DVE/GpSimd/ACT, then 4 PE
           transposes regenerate h^T into a [128, l, k, b, t16] history
           buffer that doubles as lhsT source for chunked gi1 bulk matmuls
           (layer-1 input projection) and the final output projection.
  f32r (TF32-class, 1 cycle/row at N>=512) keeps rel err ~5e-4.
"""
import json
from contextlib import ExitStack

import numpy as np

import concourse.bass as bass
import concourse.tile as tile
from concourse import mybir

f32 = mybir.dt.float32
f32r = mybir.dt.float32r
AO = mybir.AluOpType
AF = mybir.ActivationFunctionType

P = 128
B = 8           # batch per core
NCORES = 8
H = 512
G = 3 * H
KC = 4
CH = 16
LAG = 32
T_FULL = 512

# ---------------------------------------------------------------------------
# Workaround for this walrus build: it rejects >1 sync-wait per instruction.
# Split extra waits onto preceding EventSemaphore instructions on the same
# engine (same-sequencer program order preserves semantics). Hooked into
# Bass.to_json_bytes so every compile path sees compliant BIR.
_orig_to_json_bytes = bass.Bass.to_json_bytes


def _split_multiwait(mod):
    ctr = [0]

    def mk_es(engine, wait):
        ctr[0] += 1
        return {
            "debug": 0, "engine": engine, "ins": [],
            "name": f"mswsplit-{ctr[0]}", "opcode": "EventSemaphore",
            "outs": [], "sync_info": {"on_update": [], "on_wait": [wait]},
        }

    for fn in mod.get("functions", []):
        for bb in fn.get("blocks", []):
            insts = bb.get("instructions", [])
            if not any(
                len((i.get("sync_info") or {}).get("on_wait") or []) > 1
                for i in insts
            ):
                continue
            out = []
            for inst in insts:
                si = inst.get("sync_info")
                waits = (si or {}).get("on_wait") or []
                if len(waits) > 1:
                    for w in waits[:-1]:
                        out.append(mk_es(inst["engine"], w))
                    si["on_wait"] = [waits[-1]]
                out.append(inst)
            bb["instructions"] = out
    return mod


def _patched_to_json_bytes(self):
    return json.dumps(_split_multiwait(json.loads(_orig_to_json_bytes(self)))).encode()


bass.Bass.to_json_bytes = _patched_to_json_bytes


# ---------------------------------------------------------------------------
def _host_prep(inputs, core):
    x = np.ascontiguousarray(np.asarray(inputs["x"], np.float32)[core * B:(core + 1) * B])
    t = np.ascontiguousarray(np.asarray(inputs["t"], np.float32)[core * B:(core + 1) * B])
    T = x.shape[1]
    g = {k: np.asarray(v, np.float32) for k, v in inputs.items()}

    def kchunked(WT):
        F = WT.shape[1]
        return np.ascontiguousarray(
            WT.reshape(KC, P, F).transpose(1, 0, 2).reshape(P, KC * F)
        ).astype(np.float32)

    b0 = g["b_ih0"] + g["W_ih0"] @ (g["bx"] + g["bt"])
    b0 = b0.copy()
    b0[:2 * H] += g["b_hh0"][:2 * H]
    b1 = g["b_ih1"].copy()
    b1[:2 * H] += g["b_hh1"][:2 * H]
    return {
        "xT": np.ascontiguousarray(x.reshape(B * T, 64).T),
        "tT": np.ascontiguousarray(t.reshape(B * T, 1).T),
        "giA": np.ascontiguousarray((g["W_ih0"] @ g["Wx"]).T),
        "giB": np.ascontiguousarray((g["W_ih0"] @ g["Wt"]).T),
        "whhT0": kchunked(g["W_hh0"].T), "whhT1": kchunked(g["W_hh1"].T),
        "wihT1": kchunked(g["W_ih1"].T),
        "bias0": np.ascontiguousarray(np.broadcast_to(b0, (P, G))).astype(np.float32),
        "bias1": np.ascontiguousarray(np.broadcast_to(b1, (P, G))).astype(np.float32),
        "bhn0": np.ascontiguousarray(g["b_hh0"][2 * H:][None, :]),
        "bhn1": np.ascontiguousarray(g["b_hh1"][2 * H:][None, :]),
        "ones8": np.ones((1, B), np.float32),
        "id8": np.eye(B, dtype=np.float32),
        "woT": kchunked(g["Wo"].T),
        "z64": np.zeros((P, 2 * KC * B), np.float32),
        "bo_bc": np.ascontiguousarray(np.broadcast_to(g["bo"], (P, 64))).astype(np.float32),
    }


def _build(T):
    assert T % CH == 0
    NCHUNK = T // CH
    NSS = T + LAG
    ROWS = B * T

    nc = bass.Bass("TRN2", debug=False, num_devices=NCORES)

    d = {}
    d["xT"] = nc.dram_tensor("xT", [64, ROWS], f32r, kind="ExternalInput")
    d["tT"] = nc.dram_tensor("tT", [1, ROWS], f32r, kind="ExternalInput")
    d["giA"] = nc.dram_tensor("giA", [64, G], f32r, kind="ExternalInput")
    d["giB"] = nc.dram_tensor("giB", [1, G], f32r, kind="ExternalInput")
    d["whhT0"] = nc.dram_tensor("whhT0", [P, KC * G], f32r, kind="ExternalInput")
    d["whhT1"] = nc.dram_tensor("whhT1", [P, KC * G], f32r, kind="ExternalInput")
    d["wihT1"] = nc.dram_tensor("wihT1", [P, KC * G], f32r, kind="ExternalInput")
    d["bias0"] = nc.dram_tensor("bias0", [P, G], f32, kind="ExternalInput")
    d["bias1"] = nc.dram_tensor("bias1", [P, G], f32, kind="ExternalInput")
    d["bhn0"] = nc.dram_tensor("bhn0", [1, H], f32r, kind="ExternalInput")
    d["bhn1"] = nc.dram_tensor("bhn1", [1, H], f32r, kind="ExternalInput")
    d["ones8"] = nc.dram_tensor("ones8", [1, B], f32r, kind="ExternalInput")
    d["id8"] = nc.dram_tensor("id8", [B, B], f32, kind="ExternalInput")
    d["woT"] = nc.dram_tensor("woT", [P, KC * 64], f32r, kind="ExternalInput")
    d["z64"] = nc.dram_tensor("z64", [P, 2 * KC * B], f32r, kind="ExternalInput")
    d["bo_bc"] = nc.dram_tensor("bo_bc", [P, 64], f32, kind="ExternalInput")
    out_d = nc.dram_tensor("out", [B, T, 64], f32, kind="ExternalOutput")

    with tile.TileContext(nc) as tc, ExitStack() as ctx:
        wp = ctx.enter_context(tc.tile_pool(name="wp", bufs=1))
        dramp = ctx.enter_context(tc.tile_pool(name="dramp", bufs=1, space="DRAM"))

        def load(name, shape, dt):
            tl = wp.tile(shape, dt, name=f"w_{name}")
            nc.sync.dma_start(tl[:], d[name].ap())
            return tl

        whhT = [load("whhT0", [P, KC * G], f32r), load("whhT1", [P, KC * G], f32r)]
        wihT1 = load("wihT1", [P, KC * G], f32r)
        bias0 = load("bias0", [P, G], f32)
        bias1 = load("bias1", [P, G], f32)
        bhn = [load("bhn0", [1, H], f32r), load("bhn1", [1, H], f32r)]
        ones8 = load("ones8", [1, B], f32r)
        id8 = load("id8", [B, B], f32)
        woT = load("woT", [P, KC * 64], f32r)
        bo_bc = load("bo_bc", [P, 64], f32)
        giA = load("giA", [64, G], f32r)
        giB = load("giB", [1, G], f32r)

        hist_init = wp.tile([P, 2, KC, B], f32r, name="hist_init")
        nc.sync.dma_start(hist_init[:].rearrange("p a b c -> p (a b c)"), d["z64"].ap())
        hb_init = [wp.tile([B, H], f32, name=f"hb_init{l}") for l in range(2)]
        for tl in hb_init:
            nc.vector.memset(tl[:], 0.0)

        gi_d = [
            dramp.tile([B, T, G], f32, name="gi0_d"),
            dramp.tile([B, T, G], f32, name="gi1_d"),
        ]
        gi0_rows = gi_d[0][:].rearrange("b t f -> (b t) f")

        # Phase A: gi0
        with tc.tile_pool(name="pA", bufs=3) as pA, \
             tc.tile_pool(name="pAx", bufs=1) as pAx, \
             tc.tile_pool(name="psA", bufs=2, space="PSUM") as psA:
            xT_sb = pAx.tile([64, ROWS], f32r, name="xT_sb")
            nc.sync.dma_start(xT_sb[:], d["xT"].ap())
            tT_sb = pAx.tile([1, ROWS], f32r, name="tT_sb")
            nc.sync.dma_start(tT_sb[:], d["tT"].ap())
            for mt in range(ROWS // P):
                gi_sb = pA.tile([P, G], f32, name="gi0_sb")
                for gg in range(3):
                    acc = psA.tile([P, 512], f32, name="accA")
                    nc.tensor.matmul(acc[:], xT_sb[:, mt * P:(mt + 1) * P],
                                     giA[:, gg * 512:(gg + 1) * 512],
                                     start=True, stop=False)
                    nc.tensor.matmul(acc[:], tT_sb[:, mt * P:(mt + 1) * P],
                                     giB[:, gg * 512:(gg + 1) * 512],
                                     start=False, stop=True)
                    nc.vector.tensor_tensor(
                        gi_sb[:, gg * 512:(gg + 1) * 512], acc[:],
                        bias0[:, gg * 512:(gg + 1) * 512], AO.add)
                nc.sync.dma_start(gi0_rows[mt * P:(mt + 1) * P, :], gi_sb[:])

        # Phase B: recurrence
        with tc.tile_pool(name="pg", bufs=2) as pg, \
             tc.tile_pool(name="ph", bufs=2) as ph, \
             tc.tile_pool(name="pt", bufs=2) as pt, \
             tc.tile_pool(name="pb", bufs=2) as pb, \
             tc.tile_pool(name="psG", bufs=1, space="PSUM") as psG, \
             tc.tile_pool(name="psT", bufs=1, space="PSUM") as psT, \
             tc.tile_pool(name="psB", bufs=1, space="PSUM") as psB:

            hb_prev = [hb_init[0], hb_init[1]]
            hT_prev = [[hist_init[:, l, k, :] for k in range(KC)] for l in range(2)]
            hist_cur = None

            for s in range(NSS):
                act = [s < T, s >= LAG]
                t1 = s - LAG
                sidx = s % CH
                if sidx == 0:
                    hist_cur = ph.tile([P, 2, KC, B, CH], f32r, name="hist")

                gis = [None, None]
                if act[0]:
                    gis[0] = pg.tile([B, G], f32, name="gi0_t")
                    nc.sync.dma_start(gis[0][:], gi_d[0][:, s, :])
                if act[1]:
                    gis[1] = pg.tile([B, G], f32, name="gi1_t")
                    nc.sync.dma_start(gis[1][:], gi_d[1][:, t1, :])

                prz = [None, None]
                pn = [None, None]
                for l in range(2):
                    if not act[l]:
                        continue
                    prz[l] = psG.tile([B, 1024], f32, name=f"prz{l}")
                    pn[l] = psG.tile([B, 512], f32, name=f"pn{l}")
                    w = whhT[l]
                    for k in range(KC):
                        nc.tensor.matmul(prz[l][:, 0:512], hT_prev[l][k],
                                         w[:, k * G + 0:k * G + 512],
                                         start=(k == 0), stop=(k == KC - 1))
                    nc.tensor.matmul(pn[l][:], ones8[:], bhn[l][:],
                                     start=True, stop=False)
                    for k in range(KC):
                        nc.tensor.matmul(pn[l][:], hT_prev[l][k],
                                         w[:, k * G + 1024:k * G + 1536],
                                         start=False, stop=(k == KC - 1))
                    for k in range(KC):
                        nc.tensor.matmul(prz[l][:, 512:1024], hT_prev[l][k],
                                         w[:, k * G + 512:k * G + 1024],
                                         start=(k == 0), stop=(k == KC - 1))

                hb_new = [None, None]
                for l in range(2):
                    if not act[l]:
                        continue
                    g = gis[l]
                    arz = pt.tile([B, 1024], f32, name=f"arz{l}")
                    nc.vector.tensor_tensor(arz[:, 0:512], prz[l][:, 0:512],
                                            g[:, 0:512], AO.add)
                    nc.vector.tensor_tensor(arz[:, 512:1024], prz[l][:, 512:1024],
                                            g[:, 512:1024], AO.add)
                    nc.scalar.activation(arz[:], arz[:], AF.Sigmoid)
                    r = arz[:, 0:512]
                    z = arz[:, 512:1024]
                    mn = pt.tile([B, H], f32, name=f"mn{l}")
                    nc.vector.tensor_tensor(mn[:], r, pn[l][:], AO.mult)
                    nc.gpsimd.tensor_tensor(mn[:], mn[:], g[:, 1024:1536], AO.add)
                    nc.scalar.activation(mn[:], mn[:], AF.Tanh)
                    tz = pt.tile([B, H], f32, name=f"tz{l}")
                    nc.scalar.activation(tz[:], z, AF.Copy, bias=1.0, scale=-1.0)
                    w_ = pt.tile([B, H], f32, name=f"w{l}")
                    nc.gpsimd.tensor_tensor(w_[:], z, hb_prev[l][:], AO.mult)
                    hb = pt.tile([B, H], f32, name=f"hb{l}")
                    if l == 0:
                        nc.vector.tensor_tensor(hb[:], mn[:], tz[:], AO.mult)
                        nc.vector.tensor_tensor(hb[:], hb[:], w_[:], AO.add)
                    else:
                        nc.gpsimd.tensor_tensor(hb[:], mn[:], tz[:], AO.mult)
                        nc.gpsimd.tensor_tensor(hb[:], hb[:], w_[:], AO.add)
                    hb_new[l] = hb

                phT = psT.tile([P, 2, KC, B], f32, name="phT")
                for l in range(2):
                    if not act[l]:
                        continue
                    for k in range(KC):
                        nc.tensor.transpose(phT[:, l, k, :],
                                            hb_new[l][:, k * P:(k + 1) * P], id8[:])
                if act[0] and act[1]:
                    nc.scalar.copy(hist_cur[:, :, :, :, sidx], phT[:])
                else:
                    la = 0 if act[0] else 1
                    nc.scalar.copy(hist_cur[:, la, :, :, sidx], phT[:, la, :, :])

                for l in range(2):
                    if act[l]:
                        hb_prev[l] = hb_new[l]
                        hT_prev[l] = [hist_cur[:, l, k, :, sidx] for k in range(KC)]

                if sidx == CH - 1:
                    c = s // CH
                    if c < NCHUNK:
                        gi1_sb = pb.tile([P, G], f32, name="gi1_sb")
                        for gg in range(3):
                            accb = psB.tile([P, 512], f32, name="accB", tag="accB")
                            for k in range(KC):
                                nc.tensor.matmul(
                                    accb[:], hist_cur[:, 0, k, :, :],
                                    wihT1[:, k * G + gg * 512:k * G + (gg + 1) * 512],
                                    start=(k == 0), stop=(k == KC - 1))
                            nc.vector.tensor_tensor(
                                gi1_sb[:, gg * 512:(gg + 1) * 512], accb[:],
                                bias1[:, gg * 512:(gg + 1) * 512], AO.add)
                        for b_ in range(B):
                            nc.sync.dma_start(
                                gi_d[1][b_, c * CH:(c + 1) * CH, :],
                                gi1_sb[b_ * CH:(b_ + 1) * CH, :])
                    if s >= LAG + CH - 1:
                        t0 = c * CH - LAG
                        rel = pb.tile([P, KC, B, CH], f32r, name="relu_sb")
                        nc.scalar.activation(rel[:], hist_cur[:, 1, :, :, :], AF.Relu)
                        acco_full = psB.tile([P, 512], f32, name="accO", tag="accB")
                        acco = acco_full[:, 0:64]
                        for k in range(KC):
                            nc.tensor.matmul(acco[:, :], rel[:, k, :, :],
                                             woT[:, k * 64:(k + 1) * 64],
                                             start=(k == 0), stop=(k == KC - 1))
                        out_sb = pb.tile([P, 64], f32, name="out_sb")
                        nc.vector.tensor_tensor(out_sb[:], acco[:], bo_bc[:], AO.add)
                        for b_ in range(B):
                            nc.sync.dma_start(
                                out_d.ap()[b_, t0:t0 + CH, :],
                                out_sb[b_ * CH:(b_ + 1) * CH, :])
    return nc


_NC_CACHE = {}


def _get_nc(T):
    if T not in _NC_CACHE:
        _NC_CACHE[T] = _build(T)
    return _NC_CACHE[T]


def kernel(**inputs):
    from concourse.bass_utils import run_bass_kernel_spmd

    T = np.asarray(inputs["x"]).shape[1]
    nc = _get_nc(T)
    in_maps = [_host_prep(inputs, c) for c in range(NCORES)]
    res = run_bass_kernel_spmd(nc, in_maps, core_ids=list(range(NCORES)))
    out = np.concatenate([res.results[c]["out"] for c in range(NCORES)], axis=0)
    return out.astype(np.float32)


# revision 5
# speedup vs baseline: 4.4087x; 4.4087x over previous
"""Trainium2 Bass kernel for the 2-layer GRU problem (nn_GRU_43568148251487).

Contract: kernel(**inputs) takes the FULL unsharded inputs (batch 64) and
returns the FULL output [64, 512, 64]. Internally: data-parallel over batch
across 8 NeuronCores (8 sequences per core), GRU weights replicated; one
SPMD Bass program, no collectives.

Per-core structure (T=512, H=512, G=3H=1536, B=8):
  phase A: gi0 = x @ (W_ih0 Wx).T + t @ (W_ih0 Wt).T + bias  (host-fused
           input projection), bulk matmuls -> DRAM.
  phase B: 544 "supersteps": layer0 runs steps 0..511, layer1 lags by 32.
           Per step and layer: gates = h @ W_hh.T as 12 accumulating f32r
           matmuls (lhsT = transposed hidden state [128,8] chunks, rhs =
           W_hh.T [128,512] chunks streaming at 1 cycle/row), elementwise
           GRU cell on [8,512] tiles split across DVE/GpSimd/ACT, then 4 PE
           transposes regenerate h^T into a [128, l, k, b, t16] history
           buffer that doubles as lhsT source for chunked gi1 bulk matmuls
           (layer-1 input projection) and the final output projection.
  f32r (TF32-class, 1 cycle/row at N>=512) keeps rel err ~5e-4.
"""
import json
from contextlib import ExitStack

import numpy as np

import concourse.bass as bass
import concourse.tile as tile
from concourse import mybir

f32 = mybir.dt.float32
f32r = mybir.dt.float32r
AO = mybir.AluOpType
AF = mybir.ActivationFunctionType

P = 128
B = 8           # batch per core
NCORES = 8
H = 512
G = 3 * H
KC = 4
CH = 16
LAG = 32
T_FULL = 512

# ---------------------------------------------------------------------------
# Workaround for this walrus build: it rejects >1 sync-wait per instruction.
# Split extra waits onto preceding EventSemaphore instructions on the same
# engine (same-sequencer program order preserves semantics). Hooked into
# Bass.to_json_bytes so every compile path sees compliant BIR.
_orig_to_json_bytes = bass.Bass.to_json_bytes


def _split_multiwait(mod):
    ctr = [0]

    def mk_es(engine, wait):
        ctr[0] += 1
        return {
            "debug": 0, "engine": engine, "ins": [],
            "name": f"mswsplit-{ctr[0]}", "opcode": "EventSemaphore",
            "outs": [], "sync_info": {"on_update": [], "on_wait": [wait]},
        }

    for fn in mod.get("functions", []):
        for bb in fn.get("blocks", []):
            insts = bb.get("instructions", [])
            if not any(
                len((i.get("sync_info") or {}).get("on_wait") or []) > 1
                for i in insts
            ):
                continue
            out = []
            for inst in insts:
                si = inst.get("sync_info")
                waits = (si or {}).get("on_wait") or []
                if len(waits) > 1:
                    for w in waits[:-1]:
                        out.append(mk_es(inst["engine"], w))
                    si["on_wait"] = [waits[-1]]
                out.append(inst)
            bb["instructions"] = out
    return mod


def _patched_to_json_bytes(self):
    return json.dumps(_split_multiwait(json.loads(_orig_to_json_bytes(self)))).encode()


bass.Bass.to_json_bytes = _patched_to_json_bytes


# ---------------------------------------------------------------------------
def _host_prep(inputs, core):
    x = np.ascontiguousarray(np.asarray(inputs["x"], np.float32)[core * B:(core + 1) * B])
    t = np.ascontiguousarray(np.asarray(inputs["t"], np.float32)[core * B:(core + 1) * B])
    T = x.shape[1]
    g = {k: np.asarray(v, np.float32) for k, v in inputs.items()}

    def kchunked(WT):
        F = WT.shape[1]
        return np.ascontiguousarray(
            WT.reshape(KC, P, F).transpose(1, 0, 2).reshape(P, KC * F)
        ).astype(np.float32)

    b0 = g["b_ih0"] + g["W_ih0"] @ (g["bx"] + g["bt"])
    b0 = b0.copy()
    b0[:2 * H] += g["b_hh0"][:2 * H]
    b1 = g["b_ih1"].copy()
    b1[:2 * H] += g["b_hh1"][:2 * H]
    return {
        "xT": np.ascontiguousarray(x.reshape(B * T, 64).T),
        "tT": np.ascontiguousarray(t.reshape(B * T, 1).T),
        "giA": np.ascontiguousarray((g["W_ih0"] @ g["Wx"]).T),
        "giB": np.ascontiguousarray((g["W_ih0"] @ g["Wt"]).T),
        "whhT0": kchunked(g["W_hh0"].T), "whhT1": kchunked(g["W_hh1"].T),
        "wihT1": kchunked(g["W_ih1"].T),
        "bias0": np.ascontiguousarray(np.broadcast_to(b0, (P, G))).astype(np.float32),
        "bias1": np.ascontiguousarray(np.broadcast_to(b1, (P, G))).astype(np.float32),
        "bhn0": np.ascontiguousarray(g["b_hh0"][2 * H:][None, :]),
        "bhn1": np.ascontiguousarray(g["b_hh1"][2 * H:][None, :]),
        "ones8": np.ones((1, B), np.float32),
        "id8": np.eye(B, dtype=np.float32),
        "id8r": np.eye(B, dtype=np.float32),
        "woT": kchunked(g["Wo"].T),
        "z64": np.zeros((P, 2 * KC * B), np.float32),
        "bo_bc": np.ascontiguousarray(np.broadcast_to(g["bo"], (P, 64))).astype(np.float32),
    }


def _build(T):
    assert T % CH == 0
    NCHUNK = T // CH
    NSS = T + LAG
    ROWS = B * T

    nc = bass.Bass("TRN2", debug=False, num_devices=NCORES)

    d = {}
    d["xT"] = nc.dram_tensor("xT", [64, ROWS], f32r, kind="ExternalInput")
    d["tT"] = nc.dram_tensor("tT", [1, ROWS], f32r, kind="ExternalInput")
    d["giA"] = nc.dram_tensor("giA", [64, G], f32r, kind="ExternalInput")
    d["giB"] = nc.dram_tensor("giB", [1, G], f32r, kind="ExternalInput")
    d["whhT0"] = nc.dram_tensor("whhT0", [P, KC * G], f32r, kind="ExternalInput")
    d["whhT1"] = nc.dram_tensor("whhT1", [P, KC * G], f32r, kind="ExternalInput")
    d["wihT1"] = nc.dram_tensor("wihT1", [P, KC * G], f32r, kind="ExternalInput")
    d["bias0"] = nc.dram_tensor("bias0", [P, G], f32, kind="ExternalInput")
    d["bias1"] = nc.dram_tensor("bias1", [P, G], f32, kind="ExternalInput")
    d["bhn0"] = nc.dram_tensor("bhn0", [1, H], f32r, kind="ExternalInput")
    d["bhn1"] = nc.dram_tensor("bhn1", [1, H], f32r, kind="ExternalInput")
    d["ones8"] = nc.dram_tensor("ones8", [1, B], f32r, kind="ExternalInput")
    d["id8"] = nc.dram_tensor("id8", [B, B], f32, kind="ExternalInput")
    d["id8r"] = nc.dram_tensor("id8r", [B, B], f32r, kind="ExternalInput")
    d["woT"] = nc.dram_tensor("woT", [P, KC * 64], f32r, kind="ExternalInput")
    d["z64"] = nc.dram_tensor("z64", [P, 2 * KC * B], f32r, kind="ExternalInput")
    d["bo_bc"] = nc.dram_tensor("bo_bc", [P, 64], f32, kind="ExternalInput")
    out_d = nc.dram_tensor("out", [B, T, 64], f32, kind="ExternalOutput")

    with tile.TileContext(nc) as tc, ExitStack() as ctx:
        wp = ctx.enter_context(tc.tile_pool(name="wp", bufs=1))
        dramp = ctx.enter_context(tc.tile_pool(name="dramp", bufs=1, space="DRAM"))

        def load(name, shape, dt):
            tl = wp.tile(shape, dt, name=f"w_{name}")
            nc.sync.dma_start(tl[:], d[name].ap())
            return tl

        whhT = [load("whhT0", [P, KC * G], f32r), load("whhT1", [P, KC * G], f32r)]
        wihT1 = load("wihT1", [P, KC * G], f32r)
        bias0 = load("bias0", [P, G], f32)
        bias1 = load("bias1", [P, G], f32)
        bhn = [load("bhn0", [1, H], f32r), load("bhn1", [1, H], f32r)]
        ones8 = load("ones8", [1, B], f32r)
        id8 = load("id8", [B, B], f32)
        id8r = load("id8r", [B, B], f32r)
        woT = load("woT", [P, KC * 64], f32r)
        bo_bc = load("bo_bc", [P, 64], f32)
        giA = load("giA", [64, G], f32r)
        giB = load("giB", [1, G], f32r)

        hist_init = wp.tile([P, 2, KC, B], f32r, name="hist_init")
        nc.sync.dma_start(hist_init[:].rearrange("p a b c -> p (a b c)"), d["z64"].ap())
        hb_init = [wp.tile([B, H], f32, name=f"hb_init{l}") for l in range(2)]
        for tl in hb_init:
            nc.vector.memset(tl[:], 0.0)

        gi_d = [
            dramp.tile([B, T, G], f32r, name="gi0_d"),
            dramp.tile([B, T, G], f32r, name="gi1_d"),
        ]
        gi0_rows = gi_d[0][:].rearrange("b t f -> (b t) f")

        # Phase A: gi0
        with tc.tile_pool(name="pA", bufs=3) as pA, \
             tc.tile_pool(name="pAx", bufs=1) as pAx, \
             tc.tile_pool(name="psA", bufs=2, space="PSUM") as psA:
            xT_sb = pAx.tile([64, ROWS], f32r, name="xT_sb")
            nc.sync.dma_start(xT_sb[:], d["xT"].ap())
            tT_sb = pAx.tile([1, ROWS], f32r, name="tT_sb")
            nc.sync.dma_start(tT_sb[:], d["tT"].ap())
            for mt in range(ROWS // P):
                gi_sb = pA.tile([P, G], f32r, name="gi0_sb")
                for gg in range(3):
                    acc = psA.tile([P, 512], f32, name="accA")
                    nc.tensor.matmul(acc[:], xT_sb[:, mt * P:(mt + 1) * P],
                                     giA[:, gg * 512:(gg + 1) * 512],
                                     start=True, stop=False)
                    nc.tensor.matmul(acc[:], tT_sb[:, mt * P:(mt + 1) * P],
                                     giB[:, gg * 512:(gg + 1) * 512],
                                     start=False, stop=True)
                    nc.vector.tensor_tensor(
                        gi_sb[:, gg * 512:(gg + 1) * 512], acc[:],
                        bias0[:, gg * 512:(gg + 1) * 512], AO.add)
                nc.sync.dma_start(gi0_rows[mt * P:(mt + 1) * P, :], gi_sb[:])

        # Phase B: recurrence
        with tc.tile_pool(name="pg", bufs=2) as pg, \
             tc.tile_pool(name="ph", bufs=2) as ph, \
             tc.tile_pool(name="pt", bufs=2) as pt, \
             tc.tile_pool(name="pb", bufs=2) as pb, \
             tc.tile_pool(name="psG", bufs=1, space="PSUM") as psG, \
             tc.tile_pool(name="psT", bufs=1, space="PSUM") as psT, \
             tc.tile_pool(name="psB", bufs=1, space="PSUM") as psB:

            hb_prev = [hb_init[0], hb_init[1]]
            hT_prev = [[hist_init[:, l, k, :] for k in range(KC)] for l in range(2)]
            hist_cur = None

            for s in range(NSS):
                act = [s < T, s >= LAG]
                t1 = s - LAG
                sidx = s % CH
                if sidx == 0:
                    hist_cur = ph.tile([P, 2, KC, B, CH], f32r, name="hist")

                gis = [None, None]
                if act[0]:
                    gis[0] = pg.tile([B, G], f32r, name="gi0_t")
                    nc.sync.dma_start(gis[0][:], gi_d[0][:, s, :])
                if act[1]:
                    gis[1] = pg.tile([B, G], f32r, name="gi1_t")
                    nc.sync.dma_start(gis[1][:], gi_d[1][:, t1, :])

                prz = [None, None]
                pn = [None, None]
                for l in range(2):
                    if not act[l]:
                        continue
                    prz[l] = psG.tile([B, 1024], f32, name=f"prz{l}")
                    pn[l] = psG.tile([B, 512], f32, name=f"pn{l}")
                    w = whhT[l]
                    for k in range(KC):
                        nc.tensor.matmul(prz[l][:, 0:512], hT_prev[l][k],
                                         w[:, k * G + 0:k * G + 512],
                                         start=(k == 0), stop=False)
                    nc.tensor.matmul(prz[l][:, 0:512], id8r[:],
                                     gis[l][:, 0:512], start=False, stop=True)
                    nc.tensor.matmul(pn[l][:], ones8[:], bhn[l][:],
                                     start=True, stop=False)
                    for k in range(KC):
                        nc.tensor.matmul(pn[l][:], hT_prev[l][k],
                                         w[:, k * G + 1024:k * G + 1536],
                                         start=False, stop=(k == KC - 1))
                    for k in range(KC):
                        nc.tensor.matmul(prz[l][:, 512:1024], hT_prev[l][k],
                                         w[:, k * G + 512:k * G + 1024],
                                         start=(k == 0), stop=False)
                    nc.tensor.matmul(prz[l][:, 512:1024], id8r[:],
                                     gis[l][:, 512:1024], start=False, stop=True)

                hb_new = [None, None]
                for l in range(2):
                    if not act[l]:
                        continue
                    g = gis[l]
                    arz = pt.tile([B, 1024], f32, name=f"arz{l}")
                    nc.scalar.activation(arz[:], prz[l][:], AF.Sigmoid)
                    r = arz[:, 0:512]
                    z = arz[:, 512:1024]
                    mn = pt.tile([B, H], f32, name=f"mn{l}")
                    nc.vector.tensor_tensor(mn[:], r, pn[l][:], AO.mult)
                    nc.gpsimd.tensor_tensor(mn[:], mn[:], g[:, 1024:1536], AO.add)
                    nc.scalar.activation(mn[:], mn[:], AF.Tanh)
                    tz = pt.tile([B, H], f32, name=f"tz{l}")
                    nc.scalar.activation(tz[:], z, AF.Copy, bias=1.0, scale=-1.0)
                    w_ = pt.tile([B, H], f32, name=f"w{l}")
                    nc.gpsimd.tensor_tensor(w_[:], z, hb_prev[l][:], AO.mult)
                    hb = pt.tile([B, H], f32, name=f"hb{l}")
                    if l == 0:
                        nc.vector.tensor_tensor(hb[:], mn[:], tz[:], AO.mult)
                        nc.vector.tensor_tensor(hb[:], hb[:], w_[:], AO.add)
                    else:
                        nc.gpsimd.tensor_tensor(hb[:], mn[:], tz[:], AO.mult)
                        nc.gpsimd.tensor_tensor(hb[:], hb[:], w_[:], AO.add)
                    hb_new[l] = hb

                phT = psT.tile([P, 2, KC, B], f32, name="phT")
                for l in range(2):
                    if not act[l]:
                        continue
                    for k in range(KC):
                        nc.tensor.transpose(phT[:, l, k, :],
                                            hb_new[l][:, k * P:(k + 1) * P], id8[:])
                if act[0] and act[1]:
                    nc.scalar.copy(hist_cur[:, :, :, :, sidx], phT[:])
                else:
                    la = 0 if act[0] else 1
                    nc.scalar.copy(hist_cur[:, la, :, :, sidx], phT[:, la, :, :])

                for l in range(2):
                    if act[l]:
                        hb_prev[l] = hb_new[l]
                        hT_prev[l] = [hist_cur[:, l, k, :, sidx] for k in range(KC)]

                if sidx == CH - 1:
                    c = s // CH
                    if c < NCHUNK:
                        gi1_sb = pb.tile([P, G], f32r, name="gi1_sb")
                        for gg in range(3):
                            accb = psB.tile([P, 512], f32, name="accB", tag="accB")
                            for k in range(KC):
                                nc.tensor.matmul(
                                    accb[:], hist_cur[:, 0, k, :, :],
                                    wihT1[:, k * G + gg * 512:k * G + (gg + 1) * 512],
                                    start=(k == 0), stop=(k == KC - 1))
                            nc.vector.tensor_tensor(
                                gi1_sb[:, gg * 512:(gg + 1) * 512], accb[:],
                                bias1[:, gg * 512:(gg + 1) * 512], AO.add)
                        for b_ in range(B):
                            nc.sync.dma_start(
                                gi_d[1][b_, c * CH:(c + 1) * CH, :],
                                gi1_sb[b_ * CH:(b_ + 1) * CH, :])
                    if s >= LAG + CH - 1:
                        t0 = c * CH - LAG
                        rel = pb.tile([P, KC, B, CH], f32r, name="relu_sb")
                        nc.scalar.activation(rel[:], hist_cur[:, 1, :, :, :], AF.Relu)
                        acco_full = psB.tile([P, 512], f32, name="accO", tag="accB")
                        acco = acco_full[:, 0:64]
                        for k in range(KC):
                            nc.tensor.matmul(acco[:, :], rel[:, k, :, :],
                                             woT[:, k * 64:(k + 1) * 64],
                                             start=(k == 0), stop=(k == KC - 1))
                        out_sb = pb.tile([P, 64], f32, name="out_sb")
                        nc.vector.tensor_tensor(out_sb[:], acco[:], bo_bc[:], AO.add)
                        for b_ in range(B):
                            nc.sync.dma_start(
                                out_d.ap()[b_, t0:t0 + CH, :],
                                out_sb[b_ * CH:(b_ + 1) * CH, :])
    return nc


_NC_CACHE = {}


def _get_nc(T):
    if T not in _NC_CACHE:
        _NC_CACHE[T] = _build(T)
    return _NC_CACHE[T]


def kernel(**inputs):
    from concourse.bass_utils import run_bass_kernel_spmd

    T = np.asarray(inputs["x"]).shape[1]
    nc = _get_nc(T)
    in_maps = [_host_prep(inputs, c) for c in range(NCORES)]
    res = run_bass_kernel_spmd(nc, in_maps, core_ids=list(range(NCORES)))
    out = np.concatenate([res.results[c]["out"] for c in range(NCORES)], axis=0)
    return out.astype(np.float32)
